# revision 5
# baseline (speedup 1.0000x reference)
"""Causal self-attention kernel for 8 Trainium2 NeuronCores.

Sharding: core c handles batch b = c//2 and head-group hg = c%2 (8 of 16
heads). Each core computes qkv projection for its heads, head-parallel
causal attention, and a partial output projection; the host sums the two
head-group partials per batch and adds the bias terms.

On-chip layouts (per core):
  xT  [C, T]          x[b] transposed (bf16) - rhs for q/k, lhsT for v
  Q^T/K^T per pair    [128 = headA dims | headB dims, T] (bf16)
  V   per token tile  [128, 8*65] - per-head stripes [V_h | ones] so the
                      P@V matmul's ones column accumulates softmax sums
  S^T per (kt,qc)     [keys=128, queries<=512] in PSUM; exp on ScalarE
  O^T per (pair,qc)   [65, 512] PSUM accum over key tiles; row 64 = sums
  softmax denominators broadcast across partitions via K=1 f32r matmul
"""
import sys
sys.path.insert(0, '/opt/trn_rl_repo')
import numpy as np
import ml_dtypes

from concourse import bacc, mybir
import concourse.tile as tile
from concourse.bass_utils import run_bass_kernel_spmd

B, T, C, H = 4, 2048, 1024, 16
D = C // H           # 64
HPC = H // 2         # 8 heads per core
NPAIR = HPC // 2     # 4
N_CORES = 8
KC = C // 128        # 8 contraction tiles for projections
BF = mybir.dt.bfloat16
F32 = mybir.dt.float32
F32R = mybir.dt.float32r
BF_NP = ml_dtypes.bfloat16

_CACHE = {}


def build(t=T):
    QC = t // 512        # query chunks
    TT = t // 128        # token/key tiles
    nc = bacc.Bacc("TRN2", target_bir_lowering=False, debug=False,
                   num_devices=N_CORES)
    xT = nc.dram_tensor("xT", [C, t], BF, kind="ExternalInput").ap()
    wqk = nc.dram_tensor("wqk", [C, 2 * HPC * D], BF, kind="ExternalInput").ap()
    wv = nc.dram_tensor("wv", [C, HPC * D], BF, kind="ExternalInput").ap()
    wp = nc.dram_tensor("wp", [HPC * D, C], BF, kind="ExternalInput").ap()
    bq = nc.dram_tensor("bq", [128, NPAIR], F32, kind="ExternalInput").ap()
    masks = nc.dram_tensor("masks", [4 * 128, 512], BF, kind="ExternalInput").ap()
    y = nc.dram_tensor("y", [t, C], F32, kind="ExternalOutput").ap()
    masks_t = masks.rearrange("(d p) c -> d p c", p=128)

    with tile.TileContext(nc) as tc:
        with tc.tile_pool(name="const", bufs=1) as cpool, \
             tc.tile_pool(name="work", bufs=1) as wpool:
            # ---- load constants ----
            xT_sb = []
            for k in range(KC):
                tl = cpool.tile([128, t], BF, tag=f"xT{k}", name=f"xT{k}")
                nc.sync.dma_start(tl[:], xT[k * 128:(k + 1) * 128, :])
                xT_sb.append(tl)
            wqk_sb = []
            for k in range(KC):
                tl = cpool.tile([128, 2 * HPC * D], BF, tag=f"wqk{k}", name=f"wqk{k}")
                nc.sync.dma_start(tl[:], wqk[k * 128:(k + 1) * 128, :])
                wqk_sb.append(tl)
            wv_sb = []
            for k in range(KC):
                tl = cpool.tile([128, HPC * D], BF, tag=f"wv{k}", name=f"wv{k}")
                nc.sync.dma_start(tl[:], wv[k * 128:(k + 1) * 128, :])
                wv_sb.append(tl)
            wp_sb = []
            for p in range(NPAIR):
                tl = cpool.tile([128, C], BF, tag=f"wp{p}", name=f"wp{p}")
                nc.sync.dma_start(tl[:], wp[p * 128:(p + 1) * 128, :])
                wp_sb.append(tl)
            bq_sb = cpool.tile([128, NPAIR], F32, tag="bq")
            nc.sync.dma_start(bq_sb[:], bq)
            mask_sb = cpool.tile([128, 4 * 512], BF, tag="masks")
            for d in range(4):
                nc.sync.dma_start(mask_sb[:, d * 512:(d + 1) * 512], masks_t[d])
            ones32 = cpool.tile([128, 64], F32, tag="ones32")
            nc.vector.memset(ones32[:], 1.0)
            ones_sb = cpool.tile([128, 64], F32R, tag="ones")
            nc.vector.tensor_copy(ones_sb[:], ones32[:])

            # persistent intermediates
            qt_sb = [wpool.tile([128, t], BF, tag=f"qt{p}", name=f"qt{p}") for p in range(NPAIR)]
            kt_sb = [wpool.tile([128, t], BF, tag=f"kt{p}", name=f"kt{p}") for p in range(NPAIR)]
            v_sb = [wpool.tile([128, HPC * 65], BF, tag=f"v{i}", name=f"v{i}") for i in range(TT)]
            otn_sb = [wpool.tile([128, t], BF, tag=f"otn{p}", name=f"otn{p}") for p in range(NPAIR)]

            # ---- phase 1: qkv projections ----
            with tc.tile_pool(name="ps_qkv", bufs=4, space="PSUM") as ps_qkv:
                # V in natural [tokens, dims] layout (lhsT = xT tiles)
                for i in range(TT):
                    ps = ps_qkv.tile([128, HPC * D], F32, tag="ps")
                    for k in range(KC):
                        nc.tensor.matmul(ps[:], xT_sb[k][:, i * 128:(i + 1) * 128],
                                         wv_sb[k][:], start=(k == 0), stop=(k == KC - 1))
                    vt = v_sb[i][:].rearrange("p (h e) -> p h e", e=65)
                    nc.vector.memset(vt[:, :, 64:65], 1.0)
                    nc.vector.tensor_copy(
                        vt[:, :, 0:64],
                        ps[:].rearrange("p (h d) -> p h d", d=64))
                # Q^T / K^T per pair (lhsT = W tiles, rhs = xT)
                for p in range(NPAIR):
                    for mi, dst, is_q in ((2 * p, qt_sb[p], True),
                                          (2 * p + 1, kt_sb[p], False)):
                        for tch in range(t // 512):
                            ps = ps_qkv.tile([128, 512], F32, tag="ps2")
                            for k in range(KC):
                                nc.tensor.matmul(
                                    ps[:], wqk_sb[k][:, mi * 128:(mi + 1) * 128],
                                    xT_sb[k][:, tch * 512:(tch + 1) * 512],
                                    start=(k == 0), stop=(k == KC - 1))
                            if is_q:
                                nc.scalar.activation(
                                    dst[:, tch * 512:(tch + 1) * 512], ps[:],
                                    mybir.ActivationFunctionType.Identity,
                                    bias=bq_sb[:, p:p + 1])
                            else:
                                nc.vector.tensor_copy(
                                    dst[:, tch * 512:(tch + 1) * 512], ps[:])

            # ---- phase 2: attention per pair ----
            with tc.tile_pool(name="ps_s", bufs=2, space="PSUM") as psp_s, \
                 tc.tile_pool(name="ps_o", bufs=1, space="PSUM") as psp_o, \
                 tc.tile_pool(name="ps_bc", bufs=2, space="PSUM") as psp_bc, \
                 tc.tile_pool(name="att", bufs=3) as att_pool, \
                 tc.tile_pool(name="nrm", bufs=2) as nrm_pool:
                for p in range(NPAIR):
                    otnB = nrm_pool.tile([64, t], BF, tag="otnB")
                    for qc in range(QC):
                        ktmax = 4 * qc + 3
                        ps_oA = psp_o.tile([128, 512], F32, tag="oA")
                        ps_oB = psp_o.tile([128, 512], F32, tag="oB")
                        for kt in range(ktmax + 1):
                            delta = kt - 4 * qc
                            lo = 128 * delta if delta > 0 else 0
                            ps_s = psp_s.tile([128, 1024], F32, tag="s")
                            for hi, (p0, p1) in enumerate(((0, 64), (64, 128))):
                                nc.tensor.matmul(
                                    ps_s[:, hi * 512 + lo:(hi + 1) * 512],
                                    kt_sb[p][p0:p1, kt * 128:(kt + 1) * 128],
                                    qt_sb[p][p0:p1, qc * 512 + lo:(qc + 1) * 512],
                                    start=True, stop=True)
                            pt = att_pool.tile([128, 1024], BF, tag="pt")
                            nc.scalar.activation(
                                pt[:].rearrange("p (u c) -> p u c", u=2)[:, :, lo:512],
                                ps_s[:].rearrange("p (u c) -> p u c", u=2)[:, :, lo:512],
                                mybir.ActivationFunctionType.Exp)
                            if delta >= 0:
                                for hi in range(2):
                                    sl = pt[:, hi * 512 + lo:(hi + 1) * 512]
                                    nc.vector.tensor_mul(
                                        sl, sl,
                                        mask_sb[:, delta * 512 + lo:(delta + 1) * 512])
                            for hi, ps_o in ((0, ps_oA), (1, ps_oB)):
                                h = 2 * p + hi
                                nc.tensor.matmul(
                                    ps_o[0:65, lo:512],
                                    v_sb[kt][:, 65 * h:65 * h + 65],
                                    pt[:, hi * 512 + lo:(hi + 1) * 512],
                                    start=(kt == 0), stop=(kt == ktmax))
                        # normalize: recip of sums row, broadcast via K=1 matmul
                        recip = nrm_pool.tile([128, 1024], F32R, tag="recip")
                        bc_sb = nrm_pool.tile([128, 1024], F32, tag="bc")
                        for hi, ps_o in ((0, ps_oA), (1, ps_oB)):
                            with nc.allow_low_precision(
                                    reason="f32r recip feeds broadcast matmul"):
                                nc.vector.reciprocal(
                                    recip[64:65, hi * 512:(hi + 1) * 512],
                                    ps_o[64:65, 0:512])
                            ps_bc = psp_bc.tile([128, 512], F32, tag="bc")
                            nc.tensor.matmul(
                                ps_bc[0:64, :], ones_sb[64:65, 0:64],
                                recip[64:65, hi * 512:(hi + 1) * 512],
                                start=True, stop=True)
                            nc.vector.tensor_copy(
                                bc_sb[0:64, hi * 512:(hi + 1) * 512], ps_bc[0:64, :])
                        nc.vector.tensor_mul(
                            otn_sb[p][0:64, qc * 512:(qc + 1) * 512],
                            ps_oA[0:64, 0:512], bc_sb[0:64, 0:512])
                        nc.vector.tensor_mul(
                            otnB[0:64, qc * 512:(qc + 1) * 512],
                            ps_oB[0:64, 0:512], bc_sb[0:64, 512:1024])
                    # move head B rows into partitions 64:128 of otn
                    nc.sync.dma_start(otn_sb[p][64:128, :], otnB[0:64, :])

            # ---- phase 3: output projection ----
            with tc.tile_pool(name="ps_y", bufs=4, space="PSUM") as psp_y, \
                 tc.tile_pool(name="yout", bufs=4) as y_pool:
                for i in range(TT):
                    for ch in range(C // 512):
                        ps = psp_y.tile([128, 512], F32, tag="y")
                        for p in range(NPAIR):
                            nc.tensor.matmul(
                                ps[:], otn_sb[p][:, i * 128:(i + 1) * 128],
                                wp_sb[p][:, ch * 512:(ch + 1) * 512],
                                start=(p == 0), stop=(p == NPAIR - 1))
                        ysb = y_pool.tile([128, 512], F32, tag="ysb")
                        nc.vector.tensor_copy(ysb[:], ps[:])
                        nc.sync.dma_start(
                            y[i * 128:(i + 1) * 128, ch * 512:(ch + 1) * 512], ysb[:])

    nc.compile()
    return nc


def get_nc(t=T):
    if t not in _CACHE:
        _CACHE[t] = build(t)
    return _CACHE[t]


def make_masks():
    r = np.arange(128)[:, None]
    c = np.arange(512)[None, :]
    return np.concatenate(
        [(r <= c - 128 * d).astype(BF_NP) for d in range(4)], axis=0)


def prep_inputs(x, W_attn, b_attn, W_proj, t=T):
    """Per-core input maps. Core c: batch c//2, head group c%2."""
    masks = make_masks()
    qs, ks, vs = W_attn[:, :C], W_attn[:, C:2 * C], W_attn[:, 2 * C:]
    bqs = b_attn[:C]
    in_maps = []
    for core in range(N_CORES):
        b, hg = core // 2, core % 2
        xTb = np.ascontiguousarray(x[b, :t].T).astype(BF_NP)
        blocks = []
        bqcols = []
        for p in range(NPAIR):
            ha = hg * HPC + 2 * p
            qblk = qs[:, ha * D:(ha + 2) * D] * 0.125
            kblk = ks[:, ha * D:(ha + 2) * D]
            blocks += [qblk, kblk]
            bqcols.append(bqs[ha * D:(ha + 2) * D] * 0.125)
        wqk = np.concatenate(blocks, axis=1).astype(BF_NP)
        wv = vs[:, hg * HPC * D:(hg + 1) * HPC * D].astype(BF_NP)
        wp = W_proj[hg * HPC * D:(hg + 1) * HPC * D, :].astype(BF_NP)
        bq = np.stack(bqcols, axis=1).astype(np.float32)
        in_maps.append({"xT": xTb, "wqk": wqk, "wv": wv, "wp": wp,
                        "bq": bq, "masks": masks})
    return in_maps


def kernel(x, W_attn, b_attn, W_proj, b_proj):
    x = np.asarray(x, dtype=np.float32)
    W_attn = np.asarray(W_attn, dtype=np.float32)
    b_attn = np.asarray(b_attn, dtype=np.float32)
    W_proj = np.asarray(W_proj, dtype=np.float32)
    b_proj = np.asarray(b_proj, dtype=np.float32)

    nc = get_nc(T)
    in_maps = prep_inputs(x, W_attn, b_attn, W_proj, T)
    res = run_bass_kernel_spmd(nc, in_maps, list(range(N_CORES)))
    # host reduction: sum the two head-group partials; bias terms
    bias = b_attn[2 * C:] @ W_proj + b_proj  # v-bias passes through linearly
    y = np.empty((B, T, C), dtype=np.float32)
    for b in range(B):
        y[b] = res.results[2 * b]["y"] + res.results[2 * b + 1]["y"] + bias
    return y
